# revision 8
# baseline (speedup 1.0000x reference)
"""HAN layer (3-metapath GraphConv + semantic attention) on 8 trn2 NeuronCores.

Strategy (per sharding hint): shard destination nodes across the 8 cores
(6250 rows each), partition each metapath's edge list by destination shard
on the host, and sort/pad it into fixed-size 128-edge chunks per 128-dst
output tile.  Each core gathers source rows of h with indirect DMA (h is
replicated in every core's DRAM), aggregates with a selection-matrix matmul
into PSUM (z stored d-major in SBUF), computes its shard's partial semantic
attention score, all-reduces the tiny [1,3] score vector across cores,
applies softmax on-device, and writes its 6250-row slice of the output.
"""

import numpy as np

import concourse.bass as bass
import concourse.bacc as bacc
import concourse.mybir as mybir
import concourse.tile as tile
from concourse.bass_utils import run_bass_kernel_spmd

P = 128
N = 50000
D = 128
M = 3
E = 1_600_000
NCORES = 8
NSH = N // NCORES          # 6250 dst rows per core
NTILES = (NSH + P - 1) // P  # 49 output tiles (last has 106 real rows)

TRACE = False
LAST_RESULTS = None

_PROGRAM_CACHE = {}


def _preprocess(edges):
    """Host-side: per-core, per-metapath padded chunk streams.

    Returns (offs_all, dstrel_all, wgt_all, C) where each array is
    [NCORES][P, M*NTILES*C]; column (m*NTILES + t)*C + c, lane p holds edge
    (tile t, chunk c, position p) of metapath m on that core.
    """
    per_core = [[] for _ in range(NCORES)]  # per core: list over m of dicts
    counts = []
    for m in range(M):
        src = np.asarray(edges[m, 0])
        dst = np.asarray(edges[m, 1])
        out_deg = np.bincount(src, minlength=N).astype(np.float32)
        in_deg = np.bincount(dst, minlength=N).astype(np.float32)
        ns = 1.0 / np.sqrt(np.maximum(out_deg, 1.0))
        nd = 1.0 / np.sqrt(np.maximum(in_deg, 1.0))
        w_e = (ns[src] * nd[dst]).astype(np.float32)

        order = np.argsort(dst, kind="stable")
        src_s, dst_s, w_s = src[order], dst[order], w_e[order]
        shard_bounds = np.searchsorted(dst_s, NSH * np.arange(NCORES + 1))
        for core in range(NCORES):
            lo, hi = shard_bounds[core], shard_bounds[core + 1]
            sc, dc, wc = src_s[lo:hi], dst_s[lo:hi] - core * NSH, w_s[lo:hi]
            tile_id = dc >> 7  # dst_local // 128
            tile_bounds = np.searchsorted(tile_id, np.arange(NTILES + 1))
            cnts = np.diff(tile_bounds)
            counts.append(cnts)
            per_core[core].append((sc, dc, wc, tile_bounds))

    C = int(np.ceil(max(int(c.max()) for c in counts) / P))
    ncol = M * NTILES * C

    offs_all, dstrel_all, wgt_all = [], [], []
    for core in range(NCORES):
        offs = np.zeros((NTILES * M, C * P), np.int32)
        drel = np.zeros((NTILES * M, C * P), np.float32)
        wgt = np.zeros((NTILES * M, C * P), np.float32)
        for m in range(M):
            sc, dc, wc, tb = per_core[core][m]
            for t in range(NTILES):
                lo, hi = tb[t], tb[t + 1]
                n = hi - lo
                row = m * NTILES + t
                offs[row, :n] = sc[lo:hi]
                drel[row, :n] = (dc[lo:hi] - t * P).astype(np.float32)
                wgt[row, :n] = wc[lo:hi]
        # [M*NTILES, C*P] -> [M*NTILES, C, P] -> [P, M*NTILES*C]
        offs_all.append(
            np.ascontiguousarray(
                offs.reshape(ncol, P).T
            )
        )
        dstrel_all.append(np.ascontiguousarray(drel.reshape(ncol, P).T))
        wgt_all.append(np.ascontiguousarray(wgt.reshape(ncol, P).T))
    return offs_all, dstrel_all, wgt_all, C


def _build_program(C):
    if C in _PROGRAM_CACHE:
        return _PROGRAM_CACHE[C]
    f32 = mybir.dt.float32
    ncol = M * NTILES * C

    nc = bacc.Bacc(None, target_bir_lowering=False, num_devices=NCORES)
    h_d = nc.dram_tensor("h", [N, D], f32, kind="ExternalInput")
    offs_d = nc.dram_tensor("offs", [P, ncol], mybir.dt.int32, kind="ExternalInput")
    drel_d = nc.dram_tensor("drel", [P, ncol], f32, kind="ExternalInput")
    wgt_d = nc.dram_tensor("wgt", [P, ncol], f32, kind="ExternalInput")
    iota_d = nc.dram_tensor("iota", [P, P], f32, kind="ExternalInput")
    ident_d = nc.dram_tensor("ident", [P, P], f32, kind="ExternalInput")
    w1_d = nc.dram_tensor("w1", [D, D], f32, kind="ExternalInput")
    w2_d = nc.dram_tensor("w2", [D, 1], f32, kind="ExternalInput")
    b1_d = nc.dram_tensor("b1", [D, 1], f32, kind="ExternalInput")
    out_d = nc.dram_tensor("out", [NSH, D], f32, kind="ExternalOutput")
    zdump_d = nc.dram_tensor("zdump", [P, M * NTILES * P], f32, kind="ExternalOutput")
    bdump_d = nc.dram_tensor("bdump", [P, M], f32, kind="ExternalOutput")

    with tile.TileContext(nc) as tc:
        with (
            tc.tile_pool(name="consts", bufs=1) as cpool,
            tc.tile_pool(name="zpool", bufs=1) as zpool,
        ):
          with (
            tc.tile_pool(name="meta", bufs=3) as mpool,
            tc.tile_pool(name="gather", bufs=3) as gpool,
            tc.tile_pool(name="sel", bufs=4) as spool,
            tc.tile_pool(name="psum_z", bufs=2, space="PSUM") as pz,
          ):
            iota_sb = cpool.tile([P, P], f32)
            nc.sync.dma_start(out=iota_sb[:], in_=iota_d[:])
            ident_sb = cpool.tile([P, P], f32)
            nc.sync.dma_start(out=ident_sb[:], in_=ident_d[:])
            w1_sb = cpool.tile([P, P], f32)
            nc.sync.dma_start(out=w1_sb[:], in_=w1_d[:])
            w2_sb = cpool.tile([P, 1], f32)
            nc.sync.dma_start(out=w2_sb[:], in_=w2_d[:])
            b1_sb = cpool.tile([P, 1], f32)
            nc.sync.dma_start(out=b1_sb[:], in_=b1_d[:])

            # z stored d-major: z^T[m] tile t lives at columns (m*NTILES+t)*P
            z_sb = zpool.tile([P, M * NTILES * P], f32)

            # ---- sparse aggregation ----
            for m in range(M):
                for t in range(NTILES):
                    col0 = (m * NTILES + t) * C
                    o_sb = mpool.tile([P, C], mybir.dt.int32, tag="offs")
                    nc.sync.dma_start(
                        out=o_sb[:], in_=offs_d[:, col0 : col0 + C]
                    )
                    dr_sb = mpool.tile([P, C], f32, tag="drel")
                    nc.sync.dma_start(
                        out=dr_sb[:], in_=drel_d[:, col0 : col0 + C]
                    )
                    wg_sb = mpool.tile([P, C], f32, tag="wgt")
                    nc.sync.dma_start(
                        out=wg_sb[:], in_=wgt_d[:, col0 : col0 + C]
                    )
                    g_sb = gpool.tile([P, C * P], f32, tag="g")
                    for c in range(C):
                        nc.gpsimd.indirect_dma_start(
                            out=g_sb[:, c * P : (c + 1) * P],
                            out_offset=None,
                            in_=h_d[:],
                            in_offset=bass.IndirectOffsetOnAxis(
                                ap=o_sb[:, c : c + 1], axis=0
                            ),
                        )
                    psum_zt = pz.tile([P, P], f32, space="PSUM", tag="zt")
                    for c in range(C):
                        s_sb = spool.tile([P, P], f32, tag="s")
                        nc.vector.tensor_scalar(
                            out=s_sb[:],
                            in0=iota_sb[:],
                            scalar1=dr_sb[:, c : c + 1],
                            scalar2=wg_sb[:, c : c + 1],
                            op0=mybir.AluOpType.is_equal,
                            op1=mybir.AluOpType.mult,
                        )
                        nc.tensor.matmul(
                            out=psum_zt[:],
                            lhsT=g_sb[:, c * P : (c + 1) * P],
                            rhs=s_sb[:],
                            start=(c == 0),
                            stop=(c == C - 1),
                        )
                    nc.vector.tensor_copy(
                        out=z_sb[:, (m * NTILES + t) * P : (m * NTILES + t + 1) * P],
                        in_=psum_zt[:],
                    )

          # ---- semantic attention epilogue ----
          with (
            tc.tile_pool(name="epi_psum", bufs=2, space="PSUM") as ep,
            tc.tile_pool(name="score_psum", bufs=1, space="PSUM") as sp,
            tc.tile_pool(name="out_psum", bufs=2, space="PSUM") as op_,
            tc.tile_pool(name="epi_sbuf", bufs=2) as es,
            tc.tile_pool(name="small", bufs=1) as sm,
          ):
            psum_s = sp.tile([1, M * P], f32, space="PSUM")
            for m in range(M):
                for t in range(NTILES):
                    zt = z_sb[:, (m * NTILES + t) * P : (m * NTILES + t + 1) * P]
                    psum_y = ep.tile([P, P], f32, space="PSUM", tag="y")
                    nc.tensor.matmul(
                        out=psum_y[:], lhsT=w1_sb[:], rhs=zt, start=True, stop=True
                    )
                    tanh_sb = es.tile([P, P], f32, tag="tanh")
                    nc.scalar.activation(
                        out=tanh_sb[:],
                        in_=psum_y[:],
                        func=mybir.ActivationFunctionType.Tanh,
                        bias=b1_sb[:, :1],
                    )
                    nc.tensor.matmul(
                        out=psum_s[:, m * P : (m + 1) * P],
                        lhsT=w2_sb[:],
                        rhs=tanh_sb[:],
                        start=(t == 0),
                        stop=(t == NTILES - 1),
                    )
            wrow = sm.tile([1, M], f32)
            for m in range(M):
                nc.vector.reduce_sum(
                    out=wrow[:, m : m + 1],
                    in_=psum_s[:, m * P : (m + 1) * P],
                    axis=mybir.AxisListType.X,
                )
            with tc.tile_pool(name="ccdram", bufs=1, space="DRAM") as ccp:
                cc_in_t = ccp.tile([1, M], f32)
                cc_out_t = ccp.tile([1, M], f32, addr_space="Shared")
                nc.gpsimd.dma_start(cc_in_t[:], wrow[:])
                nc.gpsimd.collective_compute(
                    "AllReduce",
                    mybir.AluOpType.add,
                    replica_groups=[list(range(NCORES))],
                    ins=[cc_in_t.opt()],
                    outs=[cc_out_t.opt()],
                )
                w_bc = sm.tile([P, M], f32)
                nc.sync.dma_start(
                    out=w_bc[:], in_=cc_out_t[0:1, :].to_broadcast([P, M])
                )
            # softmax over the M columns (identical on every partition)
            nc.vector.tensor_scalar(
                out=w_bc[:],
                in0=w_bc[:],
                scalar1=1.0 / N,
                scalar2=None,
                op0=mybir.AluOpType.mult,
            )
            negmax = sm.tile([P, 1], f32)
            nc.vector.tensor_reduce(
                out=negmax[:],
                in_=w_bc[:],
                axis=mybir.AxisListType.X,
                op=mybir.AluOpType.max,
                negate=True,
            )
            e_bc = sm.tile([P, M], f32)
            nc.scalar.activation(
                out=e_bc[:],
                in_=w_bc[:],
                func=mybir.ActivationFunctionType.Exp,
                bias=negmax[:, :1],
            )
            esum = sm.tile([P, 1], f32)
            nc.vector.reduce_sum(
                out=esum[:], in_=e_bc[:], axis=mybir.AxisListType.X
            )
            rsum = sm.tile([P, 1], f32)
            nc.vector.reciprocal(out=rsum[:], in_=esum[:])
            beta = sm.tile([P, M], f32)
            nc.vector.tensor_scalar(
                out=beta[:],
                in0=e_bc[:],
                scalar1=rsum[:, :1],
                scalar2=None,
                op0=mybir.AluOpType.mult,
            )
            ibeta = sm.tile([P, M * P], f32)
            for m in range(M):
                nc.vector.tensor_scalar(
                    out=ibeta[:, m * P : (m + 1) * P],
                    in0=ident_sb[:],
                    scalar1=beta[:, m : m + 1],
                    scalar2=None,
                    op0=mybir.AluOpType.mult,
                )
            nc.sync.dma_start(out=zdump_d[:], in_=z_sb[:])
            nc.sync.dma_start(out=bdump_d[:], in_=beta[:])
            # ---- final combine: out tile = sum_m z_m^T(tile)^T @ (I * beta_m) ----
            for t in range(NTILES):
                psum_o = op_.tile([P, P], f32, space="PSUM", tag="o")
                for m in range(M):
                    nc.tensor.matmul(
                        out=psum_o[:],
                        lhsT=z_sb[:, (m * NTILES + t) * P : (m * NTILES + t + 1) * P],
                        rhs=ibeta[:, m * P : (m + 1) * P],
                        start=(m == 0),
                        stop=(m == M - 1),
                    )
                rows = min(P, NSH - t * P)
                o_sb = es.tile([P, P], f32, tag="out")
                nc.vector.tensor_copy(out=o_sb[:], in_=psum_o[:])
                nc.sync.dma_start(
                    out=out_d[t * P : t * P + rows, :], in_=o_sb[:rows, :]
                )
    nc.finalize()
    _PROGRAM_CACHE[C] = nc
    return nc


def kernel(h, edges, W1, b1, W2):
    global LAST_RESULTS
    h = np.ascontiguousarray(np.asarray(h, dtype=np.float32))
    edges = np.asarray(edges)
    offs_all, dstrel_all, wgt_all, C = _preprocess(edges)
    nc = _build_program(C)

    iota = np.tile(np.arange(P, dtype=np.float32), (P, 1))
    ident = np.eye(P, dtype=np.float32)
    w1 = np.ascontiguousarray(np.asarray(W1, dtype=np.float32))
    w2 = np.ascontiguousarray(np.asarray(W2, dtype=np.float32).reshape(D, 1))
    b1c = np.ascontiguousarray(np.asarray(b1, dtype=np.float32).reshape(D, 1))

    in_maps = []
    for core in range(NCORES):
        in_maps.append(
            {
                "h": h,
                "offs": offs_all[core],
                "drel": dstrel_all[core],
                "wgt": wgt_all[core],
                "iota": iota,
                "ident": ident,
                "w1": w1,
                "w2": w2,
                "b1": b1c,
            }
        )
    res = run_bass_kernel_spmd(
        nc, in_maps, core_ids=list(range(NCORES)), trace=TRACE
    )
    LAST_RESULTS = res
    out = np.concatenate([res.results[c]["out"] for c in range(NCORES)], axis=0)
    return out


# revision 10
# speedup vs baseline: 1091.6270x; 1091.6270x over previous
"""HAN layer (3-metapath GraphConv + semantic attention) on 8 trn2 NeuronCores.

Strategy (per sharding hint): shard destination nodes across the 8 cores
(6250 rows each), partition each metapath's edge list by destination shard
on the host, and sort/pad it into fixed-size 128-edge chunks per 128-dst
output tile.  Each core gathers source rows of h with indirect DMA (h is
replicated in every core's DRAM), aggregates with a selection-matrix matmul
into PSUM (z stored d-major in SBUF), computes its shard's partial semantic
attention score, all-reduces the tiny [1,3] score vector across cores,
applies softmax on-device, and writes its 6250-row slice of the output.
"""

import numpy as np

import concourse.bass as bass
import concourse.bacc as bacc
import concourse.mybir as mybir
import concourse.tile as tile
from concourse.bass_utils import run_bass_kernel_spmd

P = 128
N = 50000
D = 128
M = 3
E = 1_600_000
NCORES = 8
NSH = N // NCORES          # 6250 dst rows per core
NTILES = (NSH + P - 1) // P  # 49 output tiles (last has 106 real rows)

TRACE = False
LAST_RESULTS = None

_PROGRAM_CACHE = {}


def _preprocess(edges):
    """Host-side: per-core, per-metapath padded chunk streams.

    Returns (offs_all, dstrel_all, wgt_all, C) where each array is
    [NCORES][P, M*NTILES*C]; column (m*NTILES + t)*C + c, lane p holds edge
    (tile t, chunk c, position p) of metapath m on that core.
    """
    per_core = [[] for _ in range(NCORES)]  # per core: list over m of dicts
    counts = []
    for m in range(M):
        src = np.asarray(edges[m, 0])
        dst = np.asarray(edges[m, 1])
        out_deg = np.bincount(src, minlength=N).astype(np.float32)
        in_deg = np.bincount(dst, minlength=N).astype(np.float32)
        ns = 1.0 / np.sqrt(np.maximum(out_deg, 1.0))
        nd = 1.0 / np.sqrt(np.maximum(in_deg, 1.0))
        w_e = (ns[src] * nd[dst]).astype(np.float32)

        order = np.argsort(dst, kind="stable")
        src_s, dst_s, w_s = src[order], dst[order], w_e[order]
        shard_bounds = np.searchsorted(dst_s, NSH * np.arange(NCORES + 1))
        for core in range(NCORES):
            lo, hi = shard_bounds[core], shard_bounds[core + 1]
            sc, dc, wc = src_s[lo:hi], dst_s[lo:hi] - core * NSH, w_s[lo:hi]
            tile_id = dc >> 7  # dst_local // 128
            tile_bounds = np.searchsorted(tile_id, np.arange(NTILES + 1))
            cnts = np.diff(tile_bounds)
            counts.append(cnts)
            per_core[core].append((sc, dc, wc, tile_bounds))

    C = int(np.ceil(max(int(c.max()) for c in counts) / P))
    ncol = M * NTILES * C

    offs_all, dstrel_all, wgt_all = [], [], []
    for core in range(NCORES):
        offs = np.zeros((NTILES * M, C * P), np.int32)
        drel = np.zeros((NTILES * M, C * P), np.float32)
        wgt = np.zeros((NTILES * M, C * P), np.float32)
        for m in range(M):
            sc, dc, wc, tb = per_core[core][m]
            for t in range(NTILES):
                lo, hi = tb[t], tb[t + 1]
                n = hi - lo
                row = m * NTILES + t
                offs[row, :n] = sc[lo:hi]
                drel[row, :n] = (dc[lo:hi] - t * P).astype(np.float32)
                wgt[row, :n] = wc[lo:hi]
        # [M*NTILES, C*P] -> [M*NTILES, C, P] -> [P, M*NTILES*C]
        offs_all.append(
            np.ascontiguousarray(
                offs.reshape(ncol, P).T
            )
        )
        dstrel_all.append(np.ascontiguousarray(drel.reshape(ncol, P).T))
        wgt_all.append(np.ascontiguousarray(wgt.reshape(ncol, P).T))
    return offs_all, dstrel_all, wgt_all, C


def _build_program(C):
    if C in _PROGRAM_CACHE:
        return _PROGRAM_CACHE[C]
    f32 = mybir.dt.float32
    ncol = M * NTILES * C

    nc = bacc.Bacc(
        None, target_bir_lowering=False, num_devices=NCORES, num_swdge_queues=4
    )
    h_d = nc.dram_tensor("h", [N, D], f32, kind="ExternalInput")
    offs_d = nc.dram_tensor("offs", [P, ncol], mybir.dt.int32, kind="ExternalInput")
    drel_d = nc.dram_tensor("drel", [P, ncol], f32, kind="ExternalInput")
    wgt_d = nc.dram_tensor("wgt", [P, ncol], f32, kind="ExternalInput")
    iota_d = nc.dram_tensor("iota", [P, P], f32, kind="ExternalInput")
    ident_d = nc.dram_tensor("ident", [P, P], f32, kind="ExternalInput")
    w1_d = nc.dram_tensor("w1", [D, D], f32, kind="ExternalInput")
    w2_d = nc.dram_tensor("w2", [D, 1], f32, kind="ExternalInput")
    b1_d = nc.dram_tensor("b1", [D, 1], f32, kind="ExternalInput")
    out_d = nc.dram_tensor("out", [NSH, D], f32, kind="ExternalOutput")
    zdump_d = nc.dram_tensor("zdump", [P, M * NTILES * P], f32, kind="ExternalOutput")
    bdump_d = nc.dram_tensor("bdump", [P, M], f32, kind="ExternalOutput")

    with tile.TileContext(nc) as tc:
        with (
            tc.tile_pool(name="consts", bufs=1) as cpool,
            tc.tile_pool(name="zpool", bufs=1) as zpool,
        ):
          with (
            tc.tile_pool(name="meta", bufs=3) as mpool,
            tc.tile_pool(name="gather", bufs=3) as gpool,
            tc.tile_pool(name="sel", bufs=4) as spool,
            tc.tile_pool(name="psum_z", bufs=2, space="PSUM") as pz,
          ):
            iota_sb = cpool.tile([P, P], f32)
            nc.sync.dma_start(out=iota_sb[:], in_=iota_d[:])
            ident_sb = cpool.tile([P, P], f32)
            nc.sync.dma_start(out=ident_sb[:], in_=ident_d[:])
            w1_sb = cpool.tile([P, P], f32)
            nc.sync.dma_start(out=w1_sb[:], in_=w1_d[:])
            w2_sb = cpool.tile([P, 1], f32)
            nc.sync.dma_start(out=w2_sb[:], in_=w2_d[:])
            b1_sb = cpool.tile([P, 1], f32)
            nc.sync.dma_start(out=b1_sb[:], in_=b1_d[:])

            # z stored d-major: z^T[m] tile t lives at columns (m*NTILES+t)*P
            z_sb = zpool.tile([P, M * NTILES * P], f32)

            # ---- sparse aggregation ----
            for m in range(M):
                for t in range(NTILES):
                    col0 = (m * NTILES + t) * C
                    o_sb = mpool.tile([P, C], mybir.dt.int32, tag="offs")
                    nc.sync.dma_start(
                        out=o_sb[:], in_=offs_d[:, col0 : col0 + C]
                    )
                    dr_sb = mpool.tile([P, C], f32, tag="drel")
                    nc.sync.dma_start(
                        out=dr_sb[:], in_=drel_d[:, col0 : col0 + C]
                    )
                    wg_sb = mpool.tile([P, C], f32, tag="wgt")
                    nc.sync.dma_start(
                        out=wg_sb[:], in_=wgt_d[:, col0 : col0 + C]
                    )
                    g_sb = gpool.tile([P, C * P], f32, tag="g")
                    for c in range(C):
                        nc.gpsimd.indirect_dma_start(
                            out=g_sb[:, c * P : (c + 1) * P],
                            out_offset=None,
                            in_=h_d[:],
                            in_offset=bass.IndirectOffsetOnAxis(
                                ap=o_sb[:, c : c + 1], axis=0
                            ),
                        )
                    psum_zt = pz.tile([P, P], f32, space="PSUM", tag="zt")
                    for c in range(C):
                        s_sb = spool.tile([P, P], f32, tag="s")
                        nc.vector.tensor_scalar(
                            out=s_sb[:],
                            in0=iota_sb[:],
                            scalar1=dr_sb[:, c : c + 1],
                            scalar2=wg_sb[:, c : c + 1],
                            op0=mybir.AluOpType.is_equal,
                            op1=mybir.AluOpType.mult,
                        )
                        nc.tensor.matmul(
                            out=psum_zt[:],
                            lhsT=g_sb[:, c * P : (c + 1) * P],
                            rhs=s_sb[:],
                            start=(c == 0),
                            stop=(c == C - 1),
                        )
                    nc.vector.tensor_copy(
                        out=z_sb[:, (m * NTILES + t) * P : (m * NTILES + t + 1) * P],
                        in_=psum_zt[:],
                    )

          # ---- semantic attention epilogue ----
          with (
            tc.tile_pool(name="epi_psum", bufs=2, space="PSUM") as ep,
            tc.tile_pool(name="score_psum", bufs=1, space="PSUM") as sp,
            tc.tile_pool(name="out_psum", bufs=2, space="PSUM") as op_,
            tc.tile_pool(name="epi_sbuf", bufs=2) as es,
            tc.tile_pool(name="small", bufs=1) as sm,
          ):
            psum_s = sp.tile([1, M * P], f32, space="PSUM")
            for m in range(M):
                for t in range(NTILES):
                    zt = z_sb[:, (m * NTILES + t) * P : (m * NTILES + t + 1) * P]
                    psum_y = ep.tile([P, P], f32, space="PSUM", tag="y")
                    nc.tensor.matmul(
                        out=psum_y[:], lhsT=w1_sb[:], rhs=zt, start=True, stop=True
                    )
                    tanh_sb = es.tile([P, P], f32, tag="tanh")
                    nc.scalar.activation(
                        out=tanh_sb[:],
                        in_=psum_y[:],
                        func=mybir.ActivationFunctionType.Tanh,
                        bias=b1_sb[:, :1],
                    )
                    nc.tensor.matmul(
                        out=psum_s[:, m * P : (m + 1) * P],
                        lhsT=w2_sb[:],
                        rhs=tanh_sb[:],
                        start=(t == 0),
                        stop=(t == NTILES - 1),
                    )
            wrow = sm.tile([1, M], f32)
            for m in range(M):
                nc.vector.reduce_sum(
                    out=wrow[:, m : m + 1],
                    in_=psum_s[:, m * P : (m + 1) * P],
                    axis=mybir.AxisListType.X,
                )
            with tc.tile_pool(name="ccdram", bufs=1, space="DRAM") as ccp:
                cc_in_t = ccp.tile([1, M], f32)
                cc_out_t = ccp.tile([1, M], f32, addr_space="Shared")
                nc.gpsimd.dma_start(cc_in_t[:], wrow[:])
                nc.gpsimd.collective_compute(
                    "AllReduce",
                    mybir.AluOpType.add,
                    replica_groups=[list(range(NCORES))],
                    ins=[cc_in_t.opt()],
                    outs=[cc_out_t.opt()],
                )
                w_bc = sm.tile([P, M], f32)
                nc.sync.dma_start(
                    out=w_bc[:], in_=cc_out_t[0:1, :].to_broadcast([P, M])
                )
            # softmax over the M columns (identical on every partition)
            nc.vector.tensor_scalar(
                out=w_bc[:],
                in0=w_bc[:],
                scalar1=1.0 / N,
                scalar2=None,
                op0=mybir.AluOpType.mult,
            )
            negmax = sm.tile([P, 1], f32)
            nc.vector.tensor_reduce(
                out=negmax[:],
                in_=w_bc[:],
                axis=mybir.AxisListType.X,
                op=mybir.AluOpType.max,
                negate=True,
            )
            e_bc = sm.tile([P, M], f32)
            nc.scalar.activation(
                out=e_bc[:],
                in_=w_bc[:],
                func=mybir.ActivationFunctionType.Exp,
                bias=negmax[:, :1],
            )
            esum = sm.tile([P, 1], f32)
            nc.vector.reduce_sum(
                out=esum[:], in_=e_bc[:], axis=mybir.AxisListType.X
            )
            rsum = sm.tile([P, 1], f32)
            nc.vector.reciprocal(out=rsum[:], in_=esum[:])
            beta = sm.tile([P, M], f32)
            nc.vector.tensor_scalar(
                out=beta[:],
                in0=e_bc[:],
                scalar1=rsum[:, :1],
                scalar2=None,
                op0=mybir.AluOpType.mult,
            )
            ibeta = sm.tile([P, M * P], f32)
            for m in range(M):
                nc.vector.tensor_scalar(
                    out=ibeta[:, m * P : (m + 1) * P],
                    in0=ident_sb[:],
                    scalar1=beta[:, m : m + 1],
                    scalar2=None,
                    op0=mybir.AluOpType.mult,
                )
            nc.sync.dma_start(out=zdump_d[:], in_=z_sb[:])
            nc.sync.dma_start(out=bdump_d[:], in_=beta[:])
            # ---- final combine: out tile = sum_m z_m^T(tile)^T @ (I * beta_m) ----
            for t in range(NTILES):
                psum_o = op_.tile([P, P], f32, space="PSUM", tag="o")
                for m in range(M):
                    nc.tensor.matmul(
                        out=psum_o[:],
                        lhsT=z_sb[:, (m * NTILES + t) * P : (m * NTILES + t + 1) * P],
                        rhs=ibeta[:, m * P : (m + 1) * P],
                        start=(m == 0),
                        stop=(m == M - 1),
                    )
                rows = min(P, NSH - t * P)
                o_sb = es.tile([P, P], f32, tag="out")
                nc.vector.tensor_copy(out=o_sb[:], in_=psum_o[:])
                nc.sync.dma_start(
                    out=out_d[t * P : t * P + rows, :], in_=o_sb[:rows, :]
                )
    nc.finalize()
    _PROGRAM_CACHE[C] = nc
    return nc


def kernel(h, edges, W1, b1, W2):
    global LAST_RESULTS
    h = np.ascontiguousarray(np.asarray(h, dtype=np.float32))
    edges = np.asarray(edges)
    offs_all, dstrel_all, wgt_all, C = _preprocess(edges)
    nc = _build_program(C)

    iota = np.tile(np.arange(P, dtype=np.float32), (P, 1))
    ident = np.eye(P, dtype=np.float32)
    w1 = np.ascontiguousarray(np.asarray(W1, dtype=np.float32))
    w2 = np.ascontiguousarray(np.asarray(W2, dtype=np.float32).reshape(D, 1))
    b1c = np.ascontiguousarray(np.asarray(b1, dtype=np.float32).reshape(D, 1))

    in_maps = []
    for core in range(NCORES):
        in_maps.append(
            {
                "h": h,
                "offs": offs_all[core],
                "drel": dstrel_all[core],
                "wgt": wgt_all[core],
                "iota": iota,
                "ident": ident,
                "w1": w1,
                "w2": w2,
                "b1": b1c,
            }
        )
    res = run_bass_kernel_spmd(
        nc, in_maps, core_ids=list(range(NCORES)), trace=TRACE
    )
    LAST_RESULTS = res
    out = np.concatenate([res.results[c]["out"] for c in range(NCORES)], axis=0)
    return out
